# revision 1
# baseline (speedup 1.0000x reference)
"""Causal self-attention (B=8, T=1024, C=768, NH=12) on 8 TRN2 NeuronCores.

Sharding: pure data parallel — one batch element per core, no collectives.

Host side: x, w_attn, w_proj are pre-cast to bf16 (numpy/ml_dtypes) and fed
to the device program as bf16 DRAM tensors; biases stay fp32.

Per-core kernel (Bass/Tile), all intermediates resident in SBUF:
  1. xT = x.T (bf16) via PE transposes               [C=768, T=1024]
  2. qkT = w_qk-stationary matmul -> [2C, T] (q/k heads land pre-transposed
     [HD, T]); v kept row-major [T, C] with an appended ones column per head
     (v_aug [T, 12*65]) so the attention row-sums fall out of the PV matmul.
  3. Per head pair (2 heads share a 128-partition tile):
       ST = kT.T @ qT -> PSUM [tk=128, tq=512]  (2 heads packed in PE row
            groups 0-63 / 64-127, concurrent matmuls)
       causal: fully-masked blocks skipped; the diagonal-crossing 128 cols
            get an additive -1e30 triangular mask on PSUM before exp
       U  = exp(0.125 * ST) (ACT; no max-subtraction — scores are N(0,~0.3),
            bounded, exp is safe)
       yT_aug[65, tq] += v_aug.T @ U  accumulated over tk (row 64 = rowsum)
       yT = yT_aug[0:64] * (1/rowsum); the reciprocal row is broadcast
            across 64 partitions by a K=1 matmul against an ones column; the
            head1 result is written cross-half (DVE nch<=64 can shift halves).
  4. out = yT.T @ w_proj + b_proj -> [T, C] fp32, DMA out.

Matmul operands are bf16 (PE full rate, separate-LDWEIGHTS path); PSUM
accumulation is fp32. SBUF pools are never recycled (address reuse makes
later tiles inherit sync deps on DMA queues, overflowing walrus's
1-wait-per-DMA / few-waits-per-op sync budgets); PSUM pools do recycle
(compute-only accessors).
"""

import numpy as np
import ml_dtypes

import concourse.bass as bass
import concourse.bacc as bacc
import concourse.tile as tile
from concourse import mybir
from concourse.bass_utils import run_bass_kernel_spmd

B, T, C = 8, 1024, 768
NH, HD = 12, 64
P = 128
KC = C // P          # 6 k-tiles over C
KT = T // P          # 8 tiles over T
NQK = 2 * C // P     # 12 m-tiles for q+k
NHP = NH // 2        # 6 head pairs
TQB = 512            # tq block (one PSUM bank of fp32)
NB = T // TQB        # 2 tq blocks
VW = HD + 1          # 65: v columns + ones column per head

F32 = mybir.dt.float32
BF16 = mybir.dt.bfloat16
FT = mybir.ActivationFunctionType


def build_program():
    nc = bacc.Bacc("TRN2", target_bir_lowering=False, debug=False)
    xb_d = nc.dram_tensor("xb", [T, C], BF16, kind="ExternalInput").ap()
    wab_d = nc.dram_tensor("wab", [C, 3 * C], BF16, kind="ExternalInput").ap()
    ba_d = nc.dram_tensor("b_attn", [3 * C], F32, kind="ExternalInput").ap()
    wpb_d = nc.dram_tensor("wpb", [C, C], BF16, kind="ExternalInput").ap()
    bp_d = nc.dram_tensor("b_proj", [C], F32, kind="ExternalInput").ap()
    out_d = nc.dram_tensor("out", [T, C], F32, kind="ExternalOutput").ap()

    from contextlib import ExitStack

    with tile.TileContext(nc) as tc:
        with ExitStack() as ctx:
            _body(ctx, tc, xb_d, wab_d, ba_d, wpb_d, bp_d, out_d)
    nc.compile()
    return nc


def _body(ctx, tc, xb_d, wab_d, ba_d, wpb_d, bp_d, out_d):
    nc = tc.nc

    const = ctx.enter_context(tc.tile_pool(name="const", bufs=1))
    persist = ctx.enter_context(tc.tile_pool(name="persist", bufs=1))
    xin_pool = ctx.enter_context(tc.tile_pool(name="xin", bufs=8))
    wqk_pool = ctx.enter_context(tc.tile_pool(name="wqk", bufs=12))
    upool = ctx.enter_context(tc.tile_pool(name="upool", bufs=4))
    snorm = ctx.enter_context(tc.tile_pool(name="snorm", bufs=4))

    # constants ------------------------------------------------------------
    ident = const.tile([P, P], BF16)
    nc.gpsimd.memset(ident, 0.0)
    nc.gpsimd.affine_select(
        out=ident, in_=ident, compare_op=mybir.AluOpType.not_equal,
        fill=1.0, base=0, pattern=[[-1, P]], channel_multiplier=1,
    )
    # ones column at partitions 64.. for K=1 broadcast matmuls
    ones_c = const.tile([P, 64], BF16)
    nc.vector.memset(ones_c, 1.0)
    # multiplicative causal mask: 1 where tk <= tq else 0 (applied post-exp)
    tri01 = const.tile([P, P], BF16)
    nc.gpsimd.memset(tri01, 1.0)
    nc.gpsimd.affine_select(
        out=tri01, in_=tri01, compare_op=mybir.AluOpType.is_ge,
        fill=0.0, base=0, pattern=[[1, P]], channel_multiplier=-1,
    )
    # b_attn for q/k as per-partition scalars: [p, m] with b[128m + p]
    battn_pm = const.tile([P, NQK], F32)
    nc.sync.dma_start(
        out=battn_pm, in_=ba_d[0 : 2 * C].rearrange("(m p) -> p m", p=P)
    )
    # b_attn v-part / b_proj broadcast along partitions: [128, 768]
    def _pbcast(src):
        return bass.AP(tensor=src.tensor, offset=src.offset, ap=[[0, P]] + list(src.ap))

    bv_b = const.tile([P, C], F32)
    nc.sync.dma_start(out=bv_b, in_=_pbcast(ba_d[2 * C : 3 * C]))
    bp_b = const.tile([P, C], F32)
    nc.sync.dma_start(out=bp_b, in_=_pbcast(bp_d))

    # persistent SBUF tensors ---------------------------------------------
    qkT = persist.tile([P, NQK, T], BF16)        # [128, 12, 1024]  3 MB
    vaug = persist.tile([P, KT, NH * VW], BF16)  # [128, 8, 780]  1.5 MB
    yT = persist.tile([P, NHP, T], BF16)         # [128, 6, 1024] 1.5 MB
    xT = persist.tile([P, KC, T], BF16)          # [128, 6, 1024] 1.5 MB
    wv_sb = persist.tile([P, KC, C], BF16)       # [128, 6, 768]
    wp_sb = persist.tile([P, KC, C], BF16)       # [128, 6, 768]
    ot = persist.tile([P, KT, C], F32)           # [128, 8, 768]   3 MB

    x_all = persist.tile([P, KT, C], BF16, name="x_all")
    xbr = xb_d.rearrange("(t p) c -> p t c", p=P)
    for q in range(KT):
        nc.sync.dma_start(out=x_all[:, q, :], in_=xbr[:, q, :])
    for k in range(KC):
        nc.sync.dma_start(
            out=wv_sb[:, k, :], in_=wab_d[k * P : (k + 1) * P, 2 * C : 3 * C]
        )

    # ---- phase 0: transpose x into xT (bf16) ----------------------------
    with tc.tile_pool(name="tpsum", bufs=4, space="PSUM") as tpsum:
        for tt in range(KT):
            for ck in range(KC):
                pt = tpsum.tile([P, P], BF16)
                nc.tensor.transpose(
                    pt[:], x_all[:, tt, ck * P : (ck + 1) * P], ident[:]
                )
                nc.vector.tensor_copy(out=xT[:, ck, tt * P : (tt + 1) * P], in_=pt[:])

    # ---- interleaved phases 1+2 -----------------------------------------
    # v first; then per head pair: its two qk column-tiles followed by its
    # attention, so ACT/DVE overlap the next pair's QKV matmuls.
    # PSUM budget (8 banks): mmpsum 2 + spsum 4 + ypsum 2.
    with (
        tc.tile_pool(name="mmpsum", bufs=1, space="PSUM") as mmpsum,
        tc.tile_pool(name="spsum", bufs=2, space="PSUM") as spsum,
        tc.tile_pool(name="ypsum", bufs=1, space="PSUM") as ypsum,
    ):
        def qk_tile(m):
            wt = wqk_pool.tile([P, KC, P], BF16, name=f"wt{m}", tag="wt")
            nc.sync.dma_start(
                out=wt,
                in_=wab_d[:, m * P : (m + 1) * P].rearrange("(k p) n -> p k n", p=P),
            )
            ps = mmpsum.tile([P, NB, TQB], F32, name=f"qkps{m}", tag="mm")
            for n in range(NB):
                for k in range(KC):
                    nc.tensor.matmul(
                        ps[:, n, :],
                        wt[:, k, :],
                        xT[:, k, n * TQB : (n + 1) * TQB],
                        start=(k == 0),
                        stop=(k == KC - 1),
                    )
            nc.vector.tensor_tensor(
                out=qkT[:, m, :].rearrange("p (n f) -> p n f", n=NB),
                in0=ps[:, :, :],
                in1=battn_pm[:, m : m + 1, None].to_broadcast([P, NB, TQB]),
                op=mybir.AluOpType.add,
            )

        qk_tile(0)
        qk_tile(6)
        # ---- v rows (+bias), with interleaved ones cols ------------------
        vhe = vaug[:, :, :].rearrange("p t (h e) -> p t h e", e=VW)
        nc.vector.memset(vhe[:, :, :, HD : HD + 1], 1.0)
        for tt in range(KT):
            vpool = mmpsum if tt % 2 == 0 else spsum
            vtag = "mm" if tt % 2 == 0 else "pst"
            ps = vpool.tile([P, NB, TQB], F32, name=f"vps{tt}", tag=vtag)
            for n in range(NB):
                nsz = min(TQB, C - n * TQB)  # 512, 256
                for k in range(KC):
                    nc.tensor.matmul(
                        ps[:, n, :nsz],
                        xT[:, k, tt * P : (tt + 1) * P],
                        wv_sb[:, k, n * TQB : n * TQB + nsz],
                        start=(k == 0),
                        stop=(k == KC - 1),
                    )
            for n in range(NB):
                nsz = min(TQB, C - n * TQB)
                nh0 = n * TQB // HD
                nh = nsz // HD
                nc.vector.tensor_tensor(
                    out=vhe[:, tt, nh0 : nh0 + nh, 0:HD],
                    in0=ps[:, n, :nsz].rearrange("p (h e) -> p h e", e=HD),
                    in1=bv_b[:, n * TQB : n * TQB + nsz].rearrange(
                        "p (h e) -> p h e", e=HD
                    ),
                    op=mybir.AluOpType.add,
                )

        for hp in range(NHP):
            if hp > 0:
                qk_tile(hp)
                qk_tile(6 + hp)
            for b in range(NB):
                ntk = 4 * (b + 1)
                pys = [
                    ypsum.tile([P, TQB], F32, name=f"py{h}") for h in range(2)
                ]
                prev = None
                for tk in range(ntk):
                    diag = (tk // 4) == b
                    off = tk * P - b * TQB if diag else 0
                    nn = TQB - off
                    pst = spsum.tile([P, 2, TQB], F32, name="pst")
                    ut = upool.tile([P, 2, TQB], BF16, name="ut")
                    for h in range(2):
                        lo, hi = 64 * h, 64 * h + 64
                        nc.tensor.matmul(
                            pst[:, h, off:TQB],
                            qkT[lo:hi, 6 + hp, tk * P : (tk + 1) * P],
                            qkT[lo:hi, hp, b * TQB + off : (b + 1) * TQB],
                            start=True,
                            stop=True,
                        )
                    nc.scalar.activation(
                        out=ut[:, :, 0:nn],
                        in_=pst[:, :, off:TQB],
                        func=FT.Exp,
                        scale=0.125,
                    )
                    if diag:
                        nc.vector.tensor_tensor(
                            out=ut[:, :, 0:P],
                            in0=ut[:, :, 0:P],
                            in1=tri01[:, None, :].to_broadcast([P, 2, P]),
                            op=mybir.AluOpType.mult,
                        )
                    if prev is not None:
                        ptk, poff, put = prev
                        pnn = TQB - poff
                        for h in range(2):
                            nc.tensor.matmul(
                                pys[h][0:VW, poff:TQB],
                                vaug[:, ptk, (2 * hp + h) * VW : (2 * hp + h + 1) * VW],
                                put[:, h, 0:pnn],
                                start=(ptk == 0),
                                stop=False,
                            )
                    prev = (tk, off, ut)
                ptk, poff, put = prev
                pnn = TQB - poff
                for h in range(2):
                    nc.tensor.matmul(
                        pys[h][0:VW, poff:TQB],
                        vaug[:, ptk, (2 * hp + h) * VW : (2 * hp + h + 1) * VW],
                        put[:, h, 0:pnn],
                        start=(ptk == 0),
                        stop=True,
                    )
                # normalization, fully inside the pys bank: reciprocal of the
                # sums row (partition 64) into SBUF, K=1 matmul broadcast into
                # partitions 64..128 of the same bank, then y * recip -> yT.
                srow = snorm.tile([65, 2, TQB], BF16)
                with nc.allow_low_precision(reason="bf16 softmax normalization"):
                    for h in range(2):
                        nc.vector.reciprocal(
                            out=srow[64:65, h, :], in_=pys[h][64:65, :]
                        )
                sbc = snorm.tile([64, 2, TQB], F32, name="sbc")
                for h in range(2):
                    nc.tensor.matmul(
                        pys[h][64:128, :],
                        ones_c[64:65, :],
                        srow[64:65, h, :],
                        start=True,
                        stop=True,
                        tile_position=(64, 64),
                    )
                    nc.vector.tensor_copy(out=sbc[:, h, :], in_=pys[h][64:128, :])
                for h in range(2):
                    nc.vector.tensor_tensor(
                        out=yT[64 * h : 64 * h + 64, hp, b * TQB : (b + 1) * TQB],
                        in0=pys[h][0:64, :],
                        in1=sbc[:, h, :],
                        op=mybir.AluOpType.mult,
                    )

    # ---- phase 3: out = yT.T @ w_proj + b_proj --------------------------
    for k in range(KC):
        nc.sync.dma_start(out=wp_sb[:, k, :], in_=wpb_d[k * P : (k + 1) * P, :])
    with tc.tile_pool(name="opsum", bufs=2, space="PSUM") as opsum:
        for m in range(KT):
            ps = opsum.tile([P, NB, TQB], F32)
            for n in range(NB):
                nsz = min(TQB, C - n * TQB)
                for k in range(KC):
                    nc.tensor.matmul(
                        ps[:, n, :nsz],
                        yT[:, k, m * P : (m + 1) * P],
                        wp_sb[:, k, n * TQB : n * TQB + nsz],
                        start=(k == 0),
                        stop=(k == KC - 1),
                    )
            for n in range(NB):
                nsz = min(TQB, C - n * TQB)
                nc.vector.tensor_tensor(
                    out=ot[:, m, n * TQB : n * TQB + nsz],
                    in0=ps[:, n, :nsz],
                    in1=bp_b[:, n * TQB : n * TQB + nsz],
                    op=mybir.AluOpType.add,
                )
            nc.sync.dma_start(
                out=out_d.rearrange("(t p) c -> p t c", p=P)[:, m : m + 1, :],
                in_=ot[:, m : m + 1, :],
            )


_prog_cache = {}


def _get_program():
    if "nc" not in _prog_cache:
        _prog_cache["nc"] = build_program()
    return _prog_cache["nc"]


def kernel(x, w_attn, b_attn, w_proj, b_proj, _trace=False):
    nc = _get_program()
    bf = ml_dtypes.bfloat16
    xb = np.ascontiguousarray(np.asarray(x, dtype=np.float32).astype(bf))
    wab = np.ascontiguousarray(np.asarray(w_attn, dtype=np.float32).astype(bf))
    wpb = np.ascontiguousarray(np.asarray(w_proj, dtype=np.float32).astype(bf))
    b_attn = np.ascontiguousarray(np.asarray(b_attn, dtype=np.float32))
    b_proj = np.ascontiguousarray(np.asarray(b_proj, dtype=np.float32))
    in_maps = [
        {
            "xb": xb[b],
            "wab": wab,
            "b_attn": b_attn,
            "wpb": wpb,
            "b_proj": b_proj,
        }
        for b in range(B)
    ]
    res = run_bass_kernel_spmd(nc, in_maps, list(range(B)), trace=_trace)
    out = np.stack([res.results[i]["out"] for i in range(B)], axis=0)
    if _trace:
        kernel.last_results = res
    return out



# revision 8
# speedup vs baseline: 1.2503x; 1.2503x over previous
"""Causal self-attention (B=8, T=1024, C=768, NH=12) on 8 TRN2 NeuronCores.

Sharding: pure data parallel - one batch element per core, no collectives.

Host side: x, w_attn, w_proj are pre-cast to bf16 (numpy/ml_dtypes) and fed
to the device program as bf16 DRAM tensors; biases stay fp32.

Per-core kernel (Bass/Tile), all intermediates resident in SBUF:
  1. xT = x.T (bf16) via PE transposes               [C=768, T=1024]
  2. qkT = w_qk-stationary matmul -> [2C, T] (q/k heads land pre-transposed
     [HD, T]); v kept row-major [T, C] with an appended ones column per head
     (v_aug [T, 12*65]) so the attention row-sums fall out of the PV matmul.
  3. Per head pair (2 heads share a 128-partition tile), per 512-wide q block:
       ST = kT.T @ qT -> PSUM [tk=128, tq=512]  (2 heads in PE row groups)
       U  = exp(0.125 * ST) (ACT; scores are bounded, exp is safe), bf16 SBUF
       diagonal-crossing 128-col blocks get a multiplicative triangular mask
       yq[q=128, 65] += U[:, qt-block].T-as-stationary @ v_aug[tk, head]
         (U is the STATIONARY operand, v_aug [128, 65] the moving one: the
          matmul costs 65 streamed columns instead of 512, and the y output
          lands q-major so softmax normalization is a per-partition scalar)
       normalize: recip(rowsum col) on DVE, broadcast along free dim.
  4. yT = y.T via PE transposes; out = yT.T @ w_proj + b_proj -> [T, C] fp32.

Matmul operands are bf16 (PE full rate); PSUM accumulation is fp32. SBUF
pools are never recycled for DMA-written tiles (address reuse makes later
tiles inherit sync deps on DMA queues, overflowing walrus's sync budgets);
PSUM / compute-only SBUF pools do recycle.
"""

import numpy as np
import ml_dtypes

import concourse.bass as bass
import concourse.bacc as bacc
import concourse.tile as tile
from concourse import mybir
from concourse.bass_utils import run_bass_kernel_spmd

B, T, C = 8, 1024, 768
NH, HD = 12, 64
P = 128
KC = C // P          # 6 k-tiles over C
KT = T // P          # 8 tiles over T
NQK = 2 * C // P     # 12 m-tiles for q+k
NHP = NH // 2        # 6 head pairs
TQB = 512            # tq block (one PSUM bank of fp32)
NB = T // TQB        # 2 tq blocks
NQT = TQB // P       # 4 q sub-tiles of 128 per block
VW = HD + 1          # 65: v columns + ones column per head

F32 = mybir.dt.float32
BF16 = mybir.dt.bfloat16
FT = mybir.ActivationFunctionType


def build_program():
    nc = bacc.Bacc("TRN2", target_bir_lowering=False, debug=False)
    xb_d = nc.dram_tensor("xb", [T, C], BF16, kind="ExternalInput").ap()
    wab_d = nc.dram_tensor("wab", [C, 3 * C], BF16, kind="ExternalInput").ap()
    ba_d = nc.dram_tensor("b_attn", [3 * C], F32, kind="ExternalInput").ap()
    wpb_d = nc.dram_tensor("wpb", [C, C], BF16, kind="ExternalInput").ap()
    bp_d = nc.dram_tensor("b_proj", [C], F32, kind="ExternalInput").ap()
    out_d = nc.dram_tensor("out", [T, C], F32, kind="ExternalOutput").ap()

    from contextlib import ExitStack

    with tile.TileContext(nc) as tc:
        with ExitStack() as ctx:
            _body(ctx, tc, xb_d, wab_d, ba_d, wpb_d, bp_d, out_d)
    nc.compile()
    return nc


def _body(ctx, tc, xb_d, wab_d, ba_d, wpb_d, bp_d, out_d):
    nc = tc.nc

    const = ctx.enter_context(tc.tile_pool(name="const", bufs=1))
    persist = ctx.enter_context(tc.tile_pool(name="persist", bufs=1))
    wqk_pool = ctx.enter_context(tc.tile_pool(name="wqk", bufs=12))
    upool = ctx.enter_context(tc.tile_pool(name="upool", bufs=4))
    snorm = ctx.enter_context(tc.tile_pool(name="snorm", bufs=4))

    # constants ------------------------------------------------------------
    ident = const.tile([P, P], BF16)
    nc.gpsimd.memset(ident, 0.0)
    nc.gpsimd.affine_select(
        out=ident, in_=ident, compare_op=mybir.AluOpType.not_equal,
        fill=1.0, base=0, pattern=[[-1, P]], channel_multiplier=1,
    )
    # multiplicative causal mask: 1 where tk <= tq else 0 (applied post-exp)
    tri01 = const.tile([P, P], BF16)
    nc.gpsimd.memset(tri01, 1.0)
    nc.gpsimd.affine_select(
        out=tri01, in_=tri01, compare_op=mybir.AluOpType.is_ge,
        fill=0.0, base=0, pattern=[[1, P]], channel_multiplier=-1,
    )
    # b_attn for q/k as per-partition scalars: [p, m] with b[128m + p]
    battn_pm = const.tile([P, NQK], F32)
    nc.sync.dma_start(
        out=battn_pm, in_=ba_d[0 : 2 * C].rearrange("(m p) -> p m", p=P)
    )
    # b_attn v-part / b_proj broadcast along partitions: [128, 768]
    def _pbcast(src):
        return bass.AP(tensor=src.tensor, offset=src.offset, ap=[[0, P]] + list(src.ap))

    bv_b = const.tile([P, C], F32)
    nc.sync.dma_start(out=bv_b, in_=_pbcast(ba_d[2 * C : 3 * C]))
    bp_b = const.tile([P, C], F32)
    nc.sync.dma_start(out=bp_b, in_=_pbcast(bp_d))

    # persistent SBUF tensors ---------------------------------------------
    qkT = persist.tile([P, NQK, T], BF16)        # [128, 12, 1024]  3 MB
    vaug = persist.tile([P, KT, NH * VW], BF16)  # [128, 8, 780]  1.5 MB
    xT = persist.tile([P, KC, T], BF16)          # [128, 6, 1024] 1.5 MB
    wv_sb = persist.tile([P, KC, C], BF16)       # [128, 6, 768]
    wp_sb = persist.tile([P, KC, C], BF16)       # [128, 6, 768]
    # y in q-major layout: [q-part, m(8), head(12), 64] bf16
    ynorm = persist.tile([P, KT, NH, HD], BF16)  # 1.5 MB
    yTt = persist.tile([P, KC, T], BF16)         # y transposed  1.5 MB
    ot = persist.tile([P, KT, C], F32)           # [128, 8, 768]   3 MB

    x_all = persist.tile([P, KT, C], BF16, name="x_all")
    xbr = xb_d.rearrange("(t p) c -> p t c", p=P)
    for q in range(KT):
        nc.sync.dma_start(out=x_all[:, q, :], in_=xbr[:, q, :])
    for k in range(KC):
        nc.sync.dma_start(
            out=wv_sb[:, k, :], in_=wab_d[k * P : (k + 1) * P, 2 * C : 3 * C]
        )

    # ---- phase 0: transpose x into xT (bf16) ----------------------------
    with tc.tile_pool(name="tpsum", bufs=4, space="PSUM") as tpsum:
        for tt in range(KT):
            for ck in range(KC):
                pt = tpsum.tile([P, P], BF16)
                nc.tensor.transpose(
                    pt[:], x_all[:, tt, ck * P : (ck + 1) * P], ident[:]
                )
                nc.vector.tensor_copy(out=xT[:, ck, tt * P : (tt + 1) * P], in_=pt[:])

    # ---- interleaved phases 1+2 -----------------------------------------
    # v first; then per head pair: its two qk column-tiles followed by its
    # attention, so ACT/DVE overlap the next pair's QKV matmuls.
    # PSUM budget (8 banks): mm 2 (2x [128,512]) + spsum 4 (2x [128,2,512])
    # + ypsum 2 (2 head slots x [128,4,128]).
    with (
        tc.tile_pool(name="mmpsum", bufs=2, space="PSUM") as mmpsum,
        tc.tile_pool(name="spsum", bufs=2, space="PSUM") as spsum,
        tc.tile_pool(name="ypsum", bufs=1, space="PSUM") as ypsum,
    ):
        def qk_tile(m):
            wt = wqk_pool.tile([P, KC, P], BF16, name=f"wt{m}", tag="wt")
            nc.sync.dma_start(
                out=wt,
                in_=wab_d[:, m * P : (m + 1) * P].rearrange("(k p) n -> p k n", p=P),
            )
            for n in range(NB):
                ps = mmpsum.tile([P, TQB], F32, name=f"qkps{m}{n}", tag="mm")
                for k in range(KC):
                    nc.tensor.matmul(
                        ps[:],
                        wt[:, k, :],
                        xT[:, k, n * TQB : (n + 1) * TQB],
                        start=(k == 0),
                        stop=(k == KC - 1),
                    )
                nc.vector.tensor_tensor(
                    out=qkT[:, m, n * TQB : (n + 1) * TQB],
                    in0=ps[:],
                    in1=battn_pm[:, m : m + 1].to_broadcast([P, TQB]),
                    op=mybir.AluOpType.add,
                )

        qk_tile(0)
        qk_tile(6)
        # ---- v rows (+bias), with interleaved ones cols ------------------
        vhe = vaug[:, :, :].rearrange("p t (h e) -> p t h e", e=VW)
        nc.vector.memset(vhe[:, :, :, HD : HD + 1], 1.0)
        for tt in range(KT):
            for n in range(NB):
                nsz = min(TQB, C - n * TQB)  # 512, 256
                ps = mmpsum.tile([P, TQB], F32, name=f"vps{tt}{n}", tag="mm")
                for k in range(KC):
                    nc.tensor.matmul(
                        ps[:, :nsz],
                        xT[:, k, tt * P : (tt + 1) * P],
                        wv_sb[:, k, n * TQB : n * TQB + nsz],
                        start=(k == 0),
                        stop=(k == KC - 1),
                    )
                nh0 = n * TQB // HD
                nh = nsz // HD
                nc.vector.tensor_tensor(
                    out=vhe[:, tt, nh0 : nh0 + nh, 0:HD],
                    in0=ps[:, :nsz].rearrange("p (h e) -> p h e", e=HD),
                    in1=bv_b[:, n * TQB : n * TQB + nsz].rearrange(
                        "p (h e) -> p h e", e=HD
                    ),
                    op=mybir.AluOpType.add,
                )

        for hp in range(NHP):
            if hp > 0:
                qk_tile(hp)
                qk_tile(6 + hp)
            for b in range(NB):
                ntk = 4 * (b + 1)
                # y accumulators: per head one PSUM bank [128, 4, 128],
                # matmuls write [:, qt, 0:65] (col 64 = rowsum).
                pys = [
                    ypsum.tile([P, NQT, P], F32, name=f"py{h}") for h in range(2)
                ]
                prev = None

                def flush_pv(tk, off, ut):
                    # accumulation group spans the whole bank (zero region):
                    # start on the first matmul into the bank, stop on the
                    # last; the bank-wide pending-zero covers every qt slice.
                    qt_min = max(0, tk - b * NQT)
                    for h in range(2):
                        for qt in range(qt_min, NQT):
                            nc.tensor.matmul(
                                pys[h][:, qt, 0:VW],
                                ut[:, h, qt * P : (qt + 1) * P],
                                vaug[:, tk, (2 * hp + h) * VW : (2 * hp + h + 1) * VW],
                                start=(tk == 0 and qt == 0),
                                stop=(tk == ntk - 1 and qt == NQT - 1),
                            )

                for tk in range(ntk):
                    diag = (tk // NQT) == b
                    off = tk * P - b * TQB if diag else 0
                    pst = spsum.tile([P, 2, TQB], F32, name="pst")
                    ut = upool.tile([P, 2, TQB], BF16, name="ut")
                    for h in range(2):
                        lo, hi = 64 * h, 64 * h + 64
                        nc.tensor.matmul(
                            pst[:, h, off:TQB],
                            qkT[lo:hi, 6 + hp, tk * P : (tk + 1) * P],
                            qkT[lo:hi, hp, b * TQB + off : (b + 1) * TQB],
                            start=True,
                            stop=True,
                        )
                    nc.scalar.activation(
                        out=ut[:, :, off:TQB],
                        in_=pst[:, :, off:TQB],
                        func=FT.Exp,
                        scale=0.125,
                    )
                    if diag:
                        nc.vector.tensor_tensor(
                            out=ut[:, :, off : off + P],
                            in0=ut[:, :, off : off + P],
                            in1=tri01[:, None, :].to_broadcast([P, 2, P]),
                            op=mybir.AluOpType.mult,
                        )
                    if prev is not None:
                        flush_pv(*prev)
                    prev = (tk, off, ut)
                flush_pv(*prev)
                # normalization: reciprocal of the rowsum column (col 64 of
                # each q sub-tile), then multiply with free-dim broadcast.
                srec = snorm.tile([P, 2, NQT, 1], F32, name="srec")
                for h in range(2):
                    nc.vector.reciprocal(
                        out=srec[:, h, :, :], in_=pys[h][:, :, HD : HD + 1]
                    )
                for h in range(2):
                    nc.vector.tensor_tensor(
                        out=ynorm[:, b * NQT : (b + 1) * NQT, 2 * hp + h, :],
                        in0=pys[h][:, :, 0:HD],
                        in1=srec[:, h, :, :].to_broadcast([P, NQT, HD]),
                        op=mybir.AluOpType.mult,
                    )

    # ---- phase 3: yT = y.T; out = yT.T @ w_proj + b_proj ----------------
    for k in range(KC):
        nc.sync.dma_start(out=wp_sb[:, k, :], in_=wpb_d[k * P : (k + 1) * P, :])
    with (
        tc.tile_pool(name="tpsum2", bufs=4, space="PSUM") as tpsum2,
        tc.tile_pool(name="opsum", bufs=2, space="PSUM") as opsum,
    ):
        for m in range(KT):
            for k in range(KC):
                pt = tpsum2.tile([P, P], BF16)
                nc.tensor.transpose(
                    pt[:], ynorm[:, m, 2 * k : 2 * k + 2, :], ident[:]
                )
                nc.vector.tensor_copy(out=yTt[:, k, m * P : (m + 1) * P], in_=pt[:])
        for m in range(KT):
            for n in range(NB):
                nsz = min(TQB, C - n * TQB)
                ps = opsum.tile([P, TQB], F32)
                for k in range(KC):
                    nc.tensor.matmul(
                        ps[:, :nsz],
                        yTt[:, k, m * P : (m + 1) * P],
                        wp_sb[:, k, n * TQB : n * TQB + nsz],
                        start=(k == 0),
                        stop=(k == KC - 1),
                    )
                nc.vector.tensor_tensor(
                    out=ot[:, m, n * TQB : n * TQB + nsz],
                    in0=ps[:, :nsz],
                    in1=bp_b[:, n * TQB : n * TQB + nsz],
                    op=mybir.AluOpType.add,
                )
            nc.sync.dma_start(
                out=out_d.rearrange("(t p) c -> p t c", p=P)[:, m : m + 1, :],
                in_=ot[:, m : m + 1, :],
            )


_prog_cache = {}


def _get_program():
    if "nc" not in _prog_cache:
        _prog_cache["nc"] = build_program()
    return _prog_cache["nc"]


def kernel(x, w_attn, b_attn, w_proj, b_proj, _trace=False):
    nc = _get_program()
    bf = ml_dtypes.bfloat16
    xb = np.ascontiguousarray(np.asarray(x, dtype=np.float32).astype(bf))
    wab = np.ascontiguousarray(np.asarray(w_attn, dtype=np.float32).astype(bf))
    wpb = np.ascontiguousarray(np.asarray(w_proj, dtype=np.float32).astype(bf))
    b_attn = np.ascontiguousarray(np.asarray(b_attn, dtype=np.float32))
    b_proj = np.ascontiguousarray(np.asarray(b_proj, dtype=np.float32))
    in_maps = [
        {
            "xb": xb[b],
            "wab": wab,
            "b_attn": b_attn,
            "wpb": wpb,
            "b_proj": b_proj,
        }
        for b in range(B)
    ]
    res = run_bass_kernel_spmd(nc, in_maps, list(range(B)), trace=_trace)
    out = np.stack([res.results[i]["out"] for i in range(B)], axis=0)
    if _trace:
        kernel.last_results = res
    return out
